# revision 1
# baseline (speedup 1.0000x reference)
"""Blockwise-int4-dequant GEMM (BlkQ4Linear) for 8 Trainium2 NeuronCores.

Problem: out[m, n] = sum_k input[m, k] * w[n, k] + bias[n],
         w = (q_weights - q_zp) * q_scales   (block size 64 along K)
         M, K, N = 4096, 4096, 11008

Strategy (column-parallel / tensor-parallel over out_features):
  - Shard q_weights/q_scales/q_zp/bias along N across 8 cores
    (Nc = 1376 each); replicate the input; no collectives -- host
    concatenates the per-core [M, Nc] outputs.
  - Host-side layout prep only: transpose to k-major (both GEMM operands
    need K on SBUF partitions), cast input to bf16, store the centered
    codes (q - zp, integers in [-15, 15]) exactly in fp8e4m3, scales in
    bf16. The scale-dequant and the whole GEMM run on-chip.
  - Per core: dequantize w^T k-tile groups on DVE (codes * scales with
    scale rows replicated across partitions by broadcast DMA); keep the
    full dequantized w^T [K, Nc] resident in SBUF (bf16, 88KB/partition);
    stream 256-row x^T slabs; accumulate over 32 k-tiles into fp32 PSUM
    with bf16 matmuls; add bias during PSUM->SBUF eviction on DVE.
"""
import sys

for _p in ("/opt/trn_rl_repo", "/root/.axon_site/_ro/trn_rl_repo"):
    if _p not in sys.path:
        sys.path.insert(0, _p)

import numpy as np
import ml_dtypes

import concourse.bacc as bacc
import concourse.tile as tile
from concourse import mybir
from concourse.bass_utils import run_bass_kernel_spmd

BF16 = mybir.dt.bfloat16
F32 = mybir.dt.float32
FP8 = mybir.dt.float8e4
BLOCK = 64

M, K, N = 4096, 4096, 11008
NCORES = 8
NC_SHARD = N // NCORES  # 1376
MGW = 256  # m-group width (x slab)


def _n_slices(nc_width, cap=512):
    out, o = [], 0
    while o < nc_width:
        w = min(cap, nc_width - o)
        out.append((o, w))
        o += w
    return out


def build_program(M, K, Nc, MGW=256, reps=1):
    """Build + compile the per-core Bass program (identical on all cores)."""
    assert K % 128 == 0 and M % MGW == 0 and MGW % 128 == 0
    KT = K // 128
    NB = K // BLOCK
    assert NB == 2 * KT

    nc = bacc.Bacc("TRN2", target_bir_lowering=False, debug=False)

    xT = nc.dram_tensor("xT", [K, M], BF16, kind="ExternalInput")
    cT = nc.dram_tensor("cT", [K, Nc], FP8, kind="ExternalInput")
    sT = nc.dram_tensor("sT", [NB, Nc], BF16, kind="ExternalInput")
    ob = nc.dram_tensor("ob", [1, Nc], BF16, kind="ExternalInput")
    out = nc.dram_tensor("out", [M, Nc], F32, kind="ExternalOutput")

    NS = _n_slices(Nc)

    # W-prep group sizes: small first groups so wT[0] is ready fast, then
    # groups of 4 k-tiles to amortize per-DMA fixed cost.
    gsizes, rem = [], KT
    for s in (1, 1, 2):
        if rem > 0:
            gsizes.append(min(s, rem))
            rem -= gsizes[-1]
    while rem > 0:
        gsizes.append(min(4, rem))
        rem -= gsizes[-1]
    NG = len(gsizes)
    gstart = [sum(gsizes[:i]) for i in range(NG)]
    kt2g = {}
    for gi, (s, st0) in enumerate(zip(gsizes, gstart)):
        for j in range(s):
            kt2g[st0 + j] = (gi, j)
    GWMAX = 4 * Nc

    with tile.TileContext(nc) as tc:
        with (
            tc.tile_pool(name="wres", bufs=NG) as wres_pool,
            tc.tile_pool(name="const", bufs=1) as const_pool,
            tc.tile_pool(name="codes", bufs=2) as codes_pool,
            tc.tile_pool(name="srep", bufs=2) as srep_pool,
            tc.tile_pool(name="xs", bufs=2) as xs_pool,
            tc.tile_pool(name="psum", bufs=6, space="PSUM") as psum_pool,
            tc.tile_pool(name="osb", bufs=3) as osb_pool,
        ):
            # output bias replicated to all 128 partitions (eviction add)
            bias_rep = const_pool.tile([128, Nc], BF16)
            nc.scalar.dma_start(
                bias_rep[:].unsqueeze(1),
                ob[0:1, :].partition_broadcast(128),
            )

            def body():
                # --- W dequant: per group of k-tiles, fully independent ---
                wts = []
                for g in range(NG):
                    gs = gsizes[g]
                    gw = gs * Nc
                    k0 = gstart[g] * 128
                    b0 = gstart[g] * 2
                    ct = codes_pool.tile([128, GWMAX], FP8, name="ct")
                    nc.sync.dma_start(
                        ct[:, :gw].rearrange("p (j n) -> p j n", j=gs),
                        cT[k0 : k0 + gs * 128, :].rearrange(
                            "(j p) n -> p j n", p=128
                        ),
                    )
                    # scale rows -> replicated across each 64-partition half
                    st = srep_pool.tile([128, GWMAX], BF16, name="st")
                    nc.scalar.dma_start(
                        st[0:64, :gw].rearrange("p (j n) -> p j n", j=gs),
                        sT[b0 : b0 + 2 * gs : 2, :].partition_broadcast(64),
                    )
                    nc.scalar.dma_start(
                        st[64:128, :gw].rearrange("p (j n) -> p j n", j=gs),
                        sT[b0 + 1 : b0 + 2 * gs : 2, :].partition_broadcast(64),
                    )
                    wt = wres_pool.tile([128, GWMAX], BF16, name="wt")
                    nc.vector.tensor_mul(wt[:, :gw], ct[:, :gw], st[:, :gw])
                    wts.append(wt)

                # --- main GEMM ---
                for mg in range(M // MGW):
                    xs = xs_pool.tile([128, KT * MGW], BF16, name="xs")
                    xsrc = xT[:, mg * MGW : (mg + 1) * MGW].rearrange(
                        "(kt p) j -> p kt j", p=128
                    )
                    xdst = xs[:].rearrange("p (kt j) -> p kt j", kt=KT)
                    nchunk = 4 if mg == 0 else 1
                    cs = KT // nchunk
                    for ci in range(nchunk):
                        nc.sync.dma_start(
                            xdst[:, ci * cs : (ci + 1) * cs],
                            xsrc[:, ci * cs : (ci + 1) * cs],
                        )

                    NMI = MGW // 128

                    def do_mm(psums_mi, kt, mi):
                        gi, j = kt2g[kt]
                        wslice = wts[gi]
                        wo = j * Nc
                        o = kt * MGW + mi * 128
                        lhsT = xs[:, o : o + 128]
                        for si, (n0, nw) in enumerate(NS):
                            nc.tensor.matmul(
                                psums_mi[si][:, :nw],
                                lhsT,
                                wslice[:, wo + n0 : wo + n0 + nw],
                                start=(kt == 0),
                                stop=(kt == KT - 1),
                            )

                    def evict(psums_mi, mi):
                        ot = osb_pool.tile([128, Nc], F32, name="ot")
                        for si, (n0, nw) in enumerate(NS):
                            nc.vector.tensor_add(
                                ot[:, n0 : n0 + nw],
                                psums_mi[si][:, :nw],
                                bias_rep[:, n0 : n0 + nw],
                            )
                        m0 = mg * MGW + mi * 128
                        nc.scalar.dma_start(out[m0 : m0 + 128, :], ot[:])

                    if mg == 0:
                        # W window: interleave m-tiles per k-tile so PE
                        # extracts more work from each wT tile as it lands
                        psums = [
                            [
                                psum_pool.tile([128, 512], F32, name="pt")
                                for _ in NS
                            ]
                            for _ in range(NMI)
                        ]
                        for kt in range(KT):
                            for mi in range(NMI):
                                do_mm(psums[mi], kt, mi)
                        for mi in range(NMI):
                            evict(psums[mi], mi)
                    else:
                        for mi in range(NMI):
                            psums_mi = [
                                psum_pool.tile([128, 512], F32, name="pt")
                                for _ in NS
                            ]
                            for kt in range(KT):
                                do_mm(psums_mi, kt, mi)
                            evict(psums_mi, mi)

            for _ in range(reps):
                body()

    nc.compile()
    return nc


def make_in_maps(input_np, q_weights, q_scales, q_zp, bias, ncores=NCORES):
    """Host-side prep: transpose / cast / shard. Returns per-core dicts."""
    bf = ml_dtypes.bfloat16
    f8 = ml_dtypes.float8_e4m3
    n = q_weights.shape[0]
    nc_sh = n // ncores
    xT = input_np.T.astype(bf)  # [K, M] C-contiguous
    maps = []
    for i in range(ncores):
        sl = slice(i * nc_sh, (i + 1) * nc_sh)
        # centered codes: integers in [-15, 15], exact in fp8e4m3
        cc = q_weights[sl] - np.repeat(q_zp[sl], BLOCK, axis=1)
        maps.append(
            {
                "xT": xT,
                "cT": cc.T.astype(f8),
                "sT": q_scales[sl].T.astype(bf),
                "ob": bias[sl].astype(bf).reshape(1, nc_sh),
            }
        )
    return maps


_PROGRAM = None


def _get_program():
    global _PROGRAM
    if _PROGRAM is None:
        _PROGRAM = build_program(M, K, NC_SHARD, MGW)
    return _PROGRAM


def kernel(input, q_weights, q_scales, q_zp, bias):
    """Full unsharded inputs -> full [M, N] float32 output."""
    input = np.asarray(input, dtype=np.float32)
    q_weights = np.asarray(q_weights, dtype=np.int32)
    q_scales = np.asarray(q_scales, dtype=np.float32)
    q_zp = np.asarray(q_zp, dtype=np.int32)
    bias = np.asarray(bias, dtype=np.float32)

    nc = _get_program()
    maps = make_in_maps(input, q_weights, q_scales, q_zp, bias)
    res = run_bass_kernel_spmd(nc, maps, core_ids=list(range(NCORES)))
    return np.concatenate(
        [res.results[i]["out"] for i in range(NCORES)], axis=1
    )


# revision 7
# speedup vs baseline: 43.3552x; 43.3552x over previous
"""Blockwise-int4-dequant GEMM (BlkQ4Linear) for 8 Trainium2 NeuronCores.

Problem: out[m, n] = sum_k input[m, k] * w[n, k] + bias[n],
         w = (q_weights - q_zp) * q_scales   (block size 64 along K)
         M, K, N = 4096, 4096, 11008

Strategy (column-parallel / tensor-parallel over out_features):
  - Shard q_weights/q_scales/q_zp/bias along N across 8 cores
    (Nc = 1376 each); replicate the input; no collectives -- host
    concatenates the per-core [M, Nc] outputs.
  - Host-side layout prep only: transpose to k-major (both GEMM operands
    need K on SBUF partitions), cast input to bf16, store the centered
    codes (q - zp, integers in [-15, 15]) exactly in fp8e4m3, scales in
    bf16. The scale-dequant and the whole GEMM run on-chip.
  - Per core: dequantize w^T k-tile groups on DVE (codes * scales with
    scale rows replicated across partitions by broadcast DMA); keep the
    full dequantized w^T [K, Nc] resident in SBUF (bf16, 88KB/partition);
    stream 256-row x^T slabs; accumulate over 32 k-tiles into fp32 PSUM
    with bf16 matmuls; add bias during PSUM->SBUF eviction on DVE.
"""
import sys

for _p in ("/opt/trn_rl_repo", "/root/.axon_site/_ro/trn_rl_repo"):
    if _p not in sys.path:
        sys.path.insert(0, _p)

import numpy as np
import ml_dtypes

import concourse.bacc as bacc
import concourse.tile as tile
from concourse import mybir
from concourse.tile import add_dep_helper
from concourse.bass_utils import run_bass_kernel_spmd

BF16 = mybir.dt.bfloat16
F32 = mybir.dt.float32
FP8 = mybir.dt.float8e4
BLOCK = 64

M, K, N = 4096, 4096, 11008
NCORES = 8
NC_SHARD = N // NCORES  # 1376
MGW = 256  # m-group width (x slab)


def _n_slices(nc_width, cap=512):
    out, o = [], 0
    while o < nc_width:
        w = min(cap, nc_width - o)
        out.append((o, w))
        o += w
    return out


def build_program(M, K, Nc, MGW=256, reps=1):
    """Build + compile the per-core Bass program (identical on all cores)."""
    assert K % 128 == 0 and M % MGW == 0 and MGW % 128 == 0
    KT = K // 128
    NB = K // BLOCK
    assert NB == 2 * KT

    nc = bacc.Bacc("TRN2", target_bir_lowering=False, debug=False)

    xT = nc.dram_tensor("xT", [K, M], BF16, kind="ExternalInput")
    cT = nc.dram_tensor("cT", [K, Nc], FP8, kind="ExternalInput")
    sT = nc.dram_tensor("sT", [NB, Nc], BF16, kind="ExternalInput")
    ob = nc.dram_tensor("ob", [1, Nc], BF16, kind="ExternalInput")
    out = nc.dram_tensor("out", [M, Nc], F32, kind="ExternalOutput")

    NS = _n_slices(Nc)

    # W-prep group sizes: small first groups so wT[0] is ready fast, then
    # groups of 4 k-tiles to amortize per-DMA fixed cost.
    gsizes, rem = [], KT
    for s in (1, 1, 2):
        if rem > 0:
            gsizes.append(min(s, rem))
            rem -= gsizes[-1]
    while rem > 0:
        gsizes.append(min(4, rem))
        rem -= gsizes[-1]
    NG = len(gsizes)
    gstart = [sum(gsizes[:i]) for i in range(NG)]
    kt2g = {}
    for gi, (s, st0) in enumerate(zip(gsizes, gstart)):
        for j in range(s):
            kt2g[st0 + j] = (gi, j)
    GWMAX = 4 * Nc

    with tile.TileContext(nc) as tc:
        with (
            tc.tile_pool(name="wres", bufs=NG) as wres_pool,
            tc.tile_pool(name="const", bufs=1) as const_pool,
            tc.tile_pool(name="codes", bufs=2) as codes_pool,
            tc.tile_pool(name="srep", bufs=2) as srep_pool,
            tc.tile_pool(name="xs", bufs=2) as xs_pool,
            tc.tile_pool(name="psum", bufs=6, space="PSUM") as psum_pool,
            tc.tile_pool(name="osb", bufs=3) as osb_pool,
        ):
            # output bias replicated to all 128 partitions (eviction add)
            bias_rep = const_pool.tile([128, Nc], BF16)
            nc.scalar.dma_start(
                bias_rep[:].unsqueeze(1),
                ob[0:1, :].partition_broadcast(128),
            )

            def body():
                # --- W dequant: per group of k-tiles, fully independent ---
                wts = []
                wmuls = []
                for g in range(NG):
                    gs = gsizes[g]
                    gw = gs * Nc
                    k0 = gstart[g] * 128
                    b0 = gstart[g] * 2
                    ct = codes_pool.tile([128, GWMAX], FP8, name="ct")
                    nc.sync.dma_start(
                        ct[:, :gw].rearrange("p (j n) -> p j n", j=gs),
                        cT[k0 : k0 + gs * 128, :].rearrange(
                            "(j p) n -> p j n", p=128
                        ),
                    )
                    # scale rows -> replicated across each 64-partition half
                    st = srep_pool.tile([128, GWMAX], BF16, name="st")
                    nc.scalar.dma_start(
                        st[0:64, :gw].rearrange("p (j n) -> p j n", j=gs),
                        sT[b0 : b0 + 2 * gs : 2, :].partition_broadcast(64),
                    )
                    nc.scalar.dma_start(
                        st[64:128, :gw].rearrange("p (j n) -> p j n", j=gs),
                        sT[b0 + 1 : b0 + 2 * gs : 2, :].partition_broadcast(64),
                    )
                    wt = wres_pool.tile([128, GWMAX], BF16, name="wt")
                    mul_ins = nc.vector.tensor_mul(
                        wt[:, :gw], ct[:, :gw], st[:, :gw]
                    )
                    wts.append(wt)
                    wmuls.append(mul_ins)

                # --- main GEMM ---
                for mg in range(M // MGW):
                    xs = xs_pool.tile([128, KT * MGW], BF16, name="xs")
                    xsrc = xT[:, mg * MGW : (mg + 1) * MGW].rearrange(
                        "(kt p) j -> p kt j", p=128
                    )
                    xdst = xs[:].rearrange("p (kt j) -> p kt j", kt=KT)
                    nchunk = 4 if mg == 0 else 1
                    cs = KT // nchunk
                    for ci in range(nchunk):
                        d = nc.sync.dma_start(
                            xdst[:, ci * cs : (ci + 1) * cs],
                            xsrc[:, ci * cs : (ci + 1) * cs],
                        )
                        # keep early slab prefetches from competing with the
                        # W-prep DMA stream during the startup window
                        if mg in (1, 2):
                            gate = wmuls[min(2 + 2 * mg, len(wmuls) - 1)]
                            add_dep_helper(
                                d.ins, gate.ins, reason="defer xs prefetch"
                            )

                    NMI = MGW // 128

                    def do_mm(psums_mi, kt, mi):
                        gi, j = kt2g[kt]
                        wslice = wts[gi]
                        wo = j * Nc
                        o = kt * MGW + mi * 128
                        lhsT = xs[:, o : o + 128]
                        for si, (n0, nw) in enumerate(NS):
                            nc.tensor.matmul(
                                psums_mi[si][:, :nw],
                                lhsT,
                                wslice[:, wo + n0 : wo + n0 + nw],
                                start=(kt == 0),
                                stop=(kt == KT - 1),
                            )

                    def evict(psums_mi, mi):
                        ot = osb_pool.tile([128, Nc], F32, name="ot")
                        for si, (n0, nw) in enumerate(NS):
                            nc.vector.tensor_add(
                                ot[:, n0 : n0 + nw],
                                psums_mi[si][:, :nw],
                                bias_rep[:, n0 : n0 + nw],
                            )
                        m0 = mg * MGW + mi * 128
                        nc.scalar.dma_start(out[m0 : m0 + 128, :], ot[:])

                    if mg == 0:
                        # W window: interleave m-tiles per k-tile so PE
                        # extracts more work from each wT tile as it lands
                        psums = [
                            [
                                psum_pool.tile([128, 512], F32, name="pt")
                                for _ in NS
                            ]
                            for _ in range(NMI)
                        ]
                        for kt in range(KT):
                            for mi in range(NMI):
                                do_mm(psums[mi], kt, mi)
                        for mi in range(NMI):
                            evict(psums[mi], mi)
                    else:
                        for mi in range(NMI):
                            psums_mi = [
                                psum_pool.tile([128, 512], F32, name="pt")
                                for _ in NS
                            ]
                            for kt in range(KT):
                                do_mm(psums_mi, kt, mi)
                            evict(psums_mi, mi)

            for _ in range(reps):
                body()

    nc.compile()
    return nc


def make_in_maps(input_np, q_weights, q_scales, q_zp, bias, ncores=NCORES):
    """Host-side prep: transpose / cast / shard. Returns per-core dicts."""
    bf = ml_dtypes.bfloat16
    f8 = ml_dtypes.float8_e4m3
    n = q_weights.shape[0]
    nc_sh = n // ncores
    xT = input_np.T.astype(bf)  # [K, M] C-contiguous
    maps = []
    for i in range(ncores):
        sl = slice(i * nc_sh, (i + 1) * nc_sh)
        # centered codes: integers in [-15, 15], exact in fp8e4m3
        cc = q_weights[sl] - np.repeat(q_zp[sl], BLOCK, axis=1)
        maps.append(
            {
                "xT": xT,
                "cT": cc.T.astype(f8),
                "sT": q_scales[sl].T.astype(bf),
                "ob": bias[sl].astype(bf).reshape(1, nc_sh),
            }
        )
    return maps


_PROGRAM = None


def _get_program():
    global _PROGRAM
    if _PROGRAM is None:
        _PROGRAM = build_program(M, K, NC_SHARD, MGW)
    return _PROGRAM


def kernel(input, q_weights, q_scales, q_zp, bias):
    """Full unsharded inputs -> full [M, N] float32 output."""
    input = np.asarray(input, dtype=np.float32)
    q_weights = np.asarray(q_weights, dtype=np.int32)
    q_scales = np.asarray(q_scales, dtype=np.float32)
    q_zp = np.asarray(q_zp, dtype=np.int32)
    bias = np.asarray(bias, dtype=np.float32)

    nc = _get_program()
    maps = make_in_maps(input, q_weights, q_scales, q_zp, bias)
    res = run_bass_kernel_spmd(nc, maps, core_ids=list(range(NCORES)))
    return np.concatenate(
        [res.results[i]["out"] for i in range(NCORES)], axis=1
    )
